# revision 31
# baseline (speedup 1.0000x reference)
"""MetaPathTransformer Trainium2 kernel (8 NeuronCores, Bass/Tile).

Math: the reference computes heads = inv(D) @ (M0@M1@M2@M3) @ V per
(head, batch), with M_i = sum_a soft[h,a,i] * adjacency[b,a] and D the
(diagonal-by-construction) degree matrix.  The chain is reassociated
right-to-left so every step is [N,N]@[N,256] (all 8 heads' 32-wide V
blocks concatenated on the f axis); each step expands into per-relation
products A_a @ T accumulated in PSUM and combined on DVE with per-head
softmax coefficients.  inv(D) is a per-row scale.

Active design ("v2", _build_nc_v2): core c -> (b = c>>2, q = c&3),
batch x n-quarter.  Every chain step computes only the core's 256-row
quarter (T stationary, fp8 e4m3 DoubleRow, per-step power-of-two
scales folded into the combine coefficients); a 4-way 64KB AllGather
after steps 0-2 restores full-n T.  No step is computed redundantly,
so the per-core input drops to ~3.3MB (2.36MB quarter adjacency + fp8
x/Wv + f32 tail weights) vs 15.7MB for the previous f1drh layout --
the harness's single-shot measurement of f1drh (35.6us) was DMA-bound,
so input bytes are the first-order cost.  The V projection runs fp8
DoubleRow from pre-scaled x/Wv (XSC/WSC).  Adjacency streams per
relation so step 0 starts after the first 256KB.  PSUM->SBUF copies
and per-partition scalings ride the otherwise-idle ACT engine; the
per-relation combine (scalar_tensor_tensor per relation, bf16
accumulator) stays on DVE.  The last step applies inv(degree) on ACT
and feeds the W0/LN/FFN tail for the core's quarter.  kernel() folds
away the all-zero/one bias and LN vectors of setup_inputs at runtime
(checked against the actual inputs; non-trivial inputs get a fallback
build).  Numerics: ~2.9e-4 max-rel on hw (tolerance 2e-2).

Exchange notes: direct peer SBUF remote_dma_broadcast (XOR-relative
routing, slot k = quarter q^k; the trn2 die-flip x^2 on the upper quad
cancels under XOR addressing) is implemented and passes MultiCoreSim,
but the REMOTE_DMA opcodes crash the PJRT/axon runtime path on this
pod, so EXCHANGE defaults to the collective firmware ("coll").
Firmware AllGather cost is rendezvous-dominated: ~21us/call in
back-to-back rep mode, far less in a synced single shot.

The legacy f1drh build (_build_nc_legacy) is kept for comparison:
step 0 computed full-N redundantly from the whole 9.4MB adjacency,
steps 1-2 quarter + AllGather.
"""

import sys

try:
    import concourse.bass as bass  # noqa: F401
except ImportError:  # pragma: no cover
    for _p in ("/opt/trn_rl_repo", "/root/.axon_site/_ro/trn_rl_repo"):
        if _p not in sys.path:
            sys.path.insert(0, _p)
    import concourse.bass as bass  # noqa: F401

import numpy as np

import concourse.mybir as mybir
import concourse.tile as tile
from concourse import bacc
from concourse.bass import broadcast_tensor_aps
from concourse.bass_utils import run_bass_kernel_spmd

B, A, N, P, D, H = 2, 9, 1024, 4, 256, 8
DH = D // H
EPS = 1e-12
NCORES = 8
NQ = N // 4          # n-quarter per core
MC = N // 128        # m-chunks

F32 = mybir.dt.float32
F32R = mybir.dt.float32r
BF16 = mybir.dt.bfloat16
F8 = mybir.dt.float8e4
ALU = mybir.AluOpType
ACTF = mybir.ActivationFunctionType
DR = mybir.MatmulPerfMode.DoubleRow

# fp8 stage scales: T-hat_s = T_s * S[s]; adjacency is pre-scaled by 512.
# |V|max~6.7, |T1|~0.11, |T2|~0.046, |T3|~0.023 for the reference input
# distribution -> scaled maxima ~27..48, >9x margin under e4m3's 448.
SCALES = [4.0, 256.0, 1024.0, 2048.0]

MODE = "v2"      # all-quarter chain (see _build_nc_v2)
EXCHANGE = "coll"  # "coll": firmware AllGather; "rdma": peer SBUF remote-DMA
                   # (rdma crashes the PJRT/axon runtime path -- kept for
                   # environments where the SWDGE remote opcodes work)

XSC = 16.0       # x fp8 pre-scale (|x|max ~4.9 -> 78, e4m3 max 448)
WSC = 8.0        # Wv fp8 pre-scale (|Wv|max 0.153 -> 1.22)


_CACHE: dict = {}


def _build_nc(null: bool = False, reps: int = 1, no_coll: bool = False,
              no_tail: bool = False, mode: str | None = None, tt_bufs: int = 3,
              ps_bufs: int = 0, tp_bufs: int = 2, **kw):
    if mode is None:
        mode = MODE
    if mode == "v2":
        return _build_nc_v2(null=null, reps=reps, no_tail=no_tail,
                            no_coll=no_coll, tt_bufs=tt_bufs, **kw)
    return _build_nc_legacy(null=null, reps=reps, no_coll=no_coll,
                            no_tail=no_tail, mode=mode, tt_bufs=tt_bufs,
                            ps_bufs=ps_bufs, tp_bufs=tp_bufs)


def _build_nc_legacy(null: bool = False, reps: int = 1, no_coll: bool = False,
                     no_tail: bool = False, mode: str = "f1dr",
                     tt_bufs: int = 3, ps_bufs: int = 0, tp_bufs: int = 2):
    """mode: 'f2dr'  - form-2 chain, fp8 DoubleRow, broadcast-AP coef combine
             'f2drf' - form-2 chain, fp8 DoubleRow, flat full-width coef tiles
             'f2f8'  - form-2 chain, fp8 plain matmuls (FWL weight loads)
             'f1dr'  - form-1 chain (T stationary, weights reused across the 9
                       relations), fp8 DoubleRow, per-partition coef, fp8 PE
                       transposes back to [n,f] layout
    """
    nc = bacc.Bacc("TRN2", target_bir_lowering=False, debug=False,
                   num_devices=NCORES)

    dp = nc.declare_dram_parameter
    atw = N if mode in ("f1drx", "f1drh", "f1drh2") else NQ
    at_in = dp("at8", [A, 2, 2, 128, 2, atw], F8, isOutput=False)
    if mode in ("f1drx", "f1drh", "f1drh2"):
        atq_in = dp("atq8", [A, 2, 2, 128, 2, NQ], F8, isOutput=False)
    if mode == "f2drf":
        cvf_in = dp("cvf", [P * A, 128, D], F32, isOutput=False)
    if mode in ("f1dr", "f1drx", "f1drh", "f1drh2"):
        cvp_in = dp("cvp", [128, 2, P, A], F32, isOutput=False)
        id8_in = dp("ident8", [128, 128], BF16, isOutput=False)
    xt_in = dp("xt", [2, 128, N], F32R, isOutput=False)          # x[b]^T
    xtail_in = dp("xtail", [2, 128, D], F32, isOutput=False)      # quarter rows
    wv_in = dp("wv", [2, 128, D], F32R, isOutput=False)           # Wv_cat
    bv_in = dp("bv", [MC, 128, D], F32, isOutput=False)           # Bv_cat * S0
    w0_in = dp("w0", [2, 128, D], F32R, isOutput=False)
    w1_in = dp("w1", [2, 128, 2 * D], F32R, isOutput=False)
    w2_in = dp("w2", [4, 128, D], F32R, isOutput=False)
    cv_in = dp("cv", [128, P, A, H], F32, isOutput=False)         # scaled coefs
    invd_in = dp("invd", [128, 2], F32, isOutput=False)           # per-row 1/deg
    g2_in = dp("g2", [128, 2, D], F32, isOutput=False)
    b2_in = dp("b2", [128, 2, D], F32, isOutput=False)
    gf_in = dp("gf", [128, 2, 2 * D], F32, isOutput=False)
    bf_in = dp("bf", [128, 2, 2 * D], F32, isOutput=False)
    b1_in = dp("b1", [128, 2, 2 * D], F32, isOutput=False)
    b2f_in = dp("b2f", [128, D], F32, isOutput=False)
    id_in = dp("ident", [128, 128], F32R, isOutput=False)
    out_p = dp("out", [2, 128, D], F32, isOutput=True)

    ag4 = [[4 * g + i for i in range(4)] for g in range(NCORES // 4)]

    with tile.TileContext(nc) as tc:
        with (
            tc.tile_pool(name="atp", bufs=A * 4) as atp,
            tc.tile_pool(name="cst", bufs=1) as cst,
            tc.tile_pool(name="wrk", bufs=1) as wrk,
            tc.tile_pool(name="bvp", bufs=2) as bvp,
            tc.tile_pool(name="tt", bufs=tt_bufs) as tt,
            tc.tile_pool(name="ps", bufs=(ps_bufs or (3 if mode.startswith("f1") else 4)),
                         space="PSUM") as ps,
            tc.tile_pool(name="psf", bufs=1, space="PSUM") as psf,
            tc.tile_pool(name="tp", bufs=tp_bufs, space="PSUM") as tp,
            tc.tile_pool(name="tp8", bufs=1, space="PSUM") as tp8,
            tc.tile_pool(name="dram", bufs=1, space="DRAM") as dram,
        ):
            # ---- constants / small inputs ----
            ident = cst.tile([128, 128], F32R)
            nc.sync.dma_start(ident[:], id_in[:])
            cv = cst.tile([128, P, A, H], F32)
            nc.sync.dma_start(cv[:], cv_in[:])
            if mode == "f2drf":
                cvf = cst.tile([128, P * A, D], F32)
                nc.sync.dma_start(cvf[:], cvf_in.rearrange("c p f -> p c f"))
            if mode in ("f1dr", "f1drx", "f1drh", "f1drh2"):
                cvp = cst.tile([128, 2, P, A], F32)
                nc.sync.dma_start(cvp[:], cvp_in[:])
                ident8 = cst.tile([128, 128], BF16)
                nc.sync.dma_start(ident8[:], id8_in[:])
            wv = cst.tile([128, 2, D], F32R)
            nc.sync.dma_start(wv[:], wv_in.rearrange("c p f -> p c f"))
            w0 = cst.tile([128, 2, D], F32R)
            nc.sync.dma_start(w0[:], w0_in.rearrange("c p f -> p c f"))
            w1 = cst.tile([128, 2, 2 * D], F32R)
            nc.sync.dma_start(w1[:], w1_in.rearrange("c p f -> p c f"))
            w2 = cst.tile([128, 4, D], F32R)
            nc.sync.dma_start(w2[:], w2_in.rearrange("c p f -> p c f"))
            invd = cst.tile([128, 2], F32)
            nc.sync.dma_start(invd[:], invd_in[:])
            g2b = cst.tile([128, 2, D], F32)
            nc.sync.dma_start(g2b[:], g2_in[:])
            b2b = cst.tile([128, 2, D], F32)
            nc.sync.dma_start(b2b[:], b2_in[:])
            gfb = cst.tile([128, 2, 2 * D], F32)
            nc.sync.dma_start(gfb[:], gf_in[:])
            bfb = cst.tile([128, 2, 2 * D], F32)
            nc.sync.dma_start(bfb[:], bf_in[:])
            b1b = cst.tile([128, 2, 2 * D], F32)
            nc.sync.dma_start(b1b[:], b1_in[:])
            b2fb = cst.tile([128, D], F32)
            nc.sync.dma_start(b2fb[:], b2f_in[:])
            xtl = cst.tile([128, 2, D], F32)
            nc.sync.dma_start(xtl[:], xtail_in.rearrange("c p f -> p c f"))
            epst = cst.tile([128, 1], F32)
            nc.vector.memset(epst[:], EPS)

            # x^T for the V projection
            xt = wrk.tile([128, 2, N], F32R, tag="xt")
            nc.sync.dma_start(xt[:], xt_in.rearrange("c p f -> p c f"))

            # ---- adjacency^T fp8 tiles, K-plane-paired for DoubleRow ----
            at = {}
            atq = {}
            for a in range(A):
                for h in range(2):
                    for j in range(2):
                        t = atp.tile([128, 2, atw], F8, tag="AT")
                        nc.sync.dma_start(t[:], at_in[a, h, j])
                        at[(a, h, j)] = t
                        if mode in ("f1drx", "f1drh", "f1drh2"):
                            tq = atp.tile([128, 2, NQ], F8, tag="ATQ")
                            nc.sync.dma_start(tq[:], atq_in[a, h, j])
                            atq[(a, h, j)] = tq

            if null:
                zo = wrk.tile([128, D], F32, tag="ot")
                nc.vector.memset(zo[:], 0.0)
                for i in range(2):
                    nc.sync.dma_start(out_p[i], zo[:])

            def combine(s, qs, pas, dest):
                """dest <- sum_a cv[s,a,head(f)] * pas[a], via bf16 accum."""
                del qs
                acc = wrk.tile([128, D], BF16, tag="acc")
                tmp = wrk.tile([128, D], BF16, tag="tmp")
                for a in range(A):
                    if mode == "f2drf":
                        pa2, cv2 = pas[a][:], cvf[:, s * A + a, :]
                    else:
                        pa2 = pas[a][:].rearrange("p (h e) -> p h e", h=H)
                        cv3 = cv[:, s, a, :].rearrange("p (h o) -> p h o", o=1)
                        cv2, pa2 = broadcast_tensor_aps(cv3, pa2)
                    if a == 0:
                        dst = (acc[:] if mode == "f2drf" else
                               acc[:].rearrange("p (h e) -> p h e", h=H))
                        nc.vector.tensor_tensor(dst, pa2, cv2, ALU.mult)
                    else:
                        dst = (tmp[:] if mode == "f2drf" else
                               tmp[:].rearrange("p (h e) -> p h e", h=H))
                        nc.vector.tensor_tensor(dst, pa2, cv2, ALU.mult)
                        if a < A - 1:
                            nc.vector.tensor_add(acc[:], acc[:], tmp[:])
                        else:
                            nc.vector.tensor_add(dest, acc[:], tmp[:])

            for rep in range(0 if null else reps):
                # ---- V = (x @ Wv_cat + Bv) * S0 -> T0 fp8 ----
                tcur = tt.tile([128, 2, 2, 2, D], F8, tag="T")
                for m in range(MC):
                    pv = ps.tile([128, D], F32, tag="pa")
                    for dc in range(2):
                        nc.tensor.matmul(
                            pv[:], xt[:, dc, m * 128:(m + 1) * 128], wv[:, dc, :],
                            start=(dc == 0), stop=(dc == 1))
                    bvt = bvp.tile([128, D], F32, tag="bv")
                    nc.sync.dma_start(bvt[:], bv_in[m])
                    h, j, pl = m & 1, m >> 2, (m >> 1) & 1
                    nc.vector.scalar_tensor_tensor(
                        tcur[:, h, j, pl, :], pv[:], SCALES[0], bvt[:],
                        op0=ALU.mult, op1=ALU.add)

                # ---- chain: 4 steps of T <- sum_a c[s,a] (*) (A_a @ T) ----
                attn = None
                for s in range(P):
                    if s < P - 1:
                        accr8 = wrk.tile([128, 2, D], F8, tag="accr8")
                    else:
                        attn = wrk.tile([128, 2, D], F32R, tag="attn")

                    if (mode == "f1drx" or (mode == "f1drh" and s == 0)
                            or (mode == "f1drh2" and s <= 1)):
                        # exchange-free: every core redundantly computes the
                        # full-N T update for steps 0..P-2 (it holds the whole
                        # batch adjacency in fp8), only its quarter for the
                        # last step.  No collectives anywhere in the chain.
                        last = s == P - 1
                        qw = NQ if last else N
                        att = atq if last else at
                        tnext = None
                        if not last:
                            tnext = tt.tile([128, 2, 2, 2, D], F8, tag="T")
                        for fc2 in range(2):
                            fsl = slice(fc2 * 128, (fc2 + 1) * 128)
                            acc = wrk.tile([128, qw], F32R, tag="accp")
                            a8 = wrk.tile([128, qw], BF16, tag="a8")
                            for qh in range(qw // 512 or 1):
                                qsl = slice(qh * 512,
                                            min(qw, (qh + 1) * 512))
                                osl = slice(qh * 512, min(qw, (qh + 1) * 512))
                                ow = osl.stop - osl.start
                                for a in range(A):
                                    pa = ps.tile([128, 512], F32, tag="pa")
                                    k = 0
                                    for hh in range(2):
                                        for jj in range(2):
                                            nc.tensor.matmul(
                                                pa[:, :ow],
                                                tcur[:, hh, jj, :, fsl],
                                                att[(a, hh, jj)][:, :, qsl],
                                                start=(k == 0), stop=(k == 3),
                                                perf_mode=DR)
                                            k += 1
                                    sc = cvp[:, fc2, s, a:a + 1]
                                    if a == 0:
                                        nc.vector.tensor_scalar_mul(
                                            acc[:, osl], pa[:, :ow], sc)
                                    elif not last and a == A - 1:
                                        nc.vector.scalar_tensor_tensor(
                                            a8[:, osl], pa[:, :ow], sc,
                                            acc[:, osl],
                                            op0=ALU.mult, op1=ALU.add)
                                    else:
                                        nc.vector.scalar_tensor_tensor(
                                            acc[:, osl], pa[:, :ow], sc,
                                            acc[:, osl],
                                            op0=ALU.mult, op1=ALU.add)
                            if not last:
                                for mq in range(MC):
                                    pt8 = tp8.tile([128, 128], BF16, tag="tp8")
                                    nc.tensor.transpose(
                                        pt8[:], a8[:, mq * 128:(mq + 1) * 128],
                                        ident8[:])
                                    nc.vector.tensor_copy(
                                        tnext[:, mq & 1, mq >> 2,
                                              (mq >> 1) & 1, fsl], pt8[:])
                            else:
                                for qs in range(2):
                                    ptr = tp.tile([128, 128], F32R, tag="tp")
                                    nc.tensor.transpose(
                                        ptr[:], acc[:, qs * 128:(qs + 1) * 128],
                                        ident[:])
                                    nc.vector.tensor_scalar_mul(
                                        attn[:, qs, fsl], ptr[:],
                                        invd[:, qs:qs + 1])
                        if not last:
                            tcur = tnext
                        continue

                    if mode in ("f1dr", "f1drh", "f1drh2"):
                        att1 = at if mode == "f1dr" else atq
                        # T stationary: one weight load serves all 9 relations;
                        # output lands [f-half, q] and is PE-transposed back.
                        for fc2 in range(2):
                            fsl = slice(fc2 * 128, (fc2 + 1) * 128)
                            acc = wrk.tile([128, D], F32R, tag="accp")
                            a8 = wrk.tile([128, D], BF16, tag="a8")
                            for a in range(A):
                                pa = ps.tile([128, D], F32, tag="pa")
                                k = 0
                                for hh in range(2):
                                    for jj in range(2):
                                        nc.tensor.matmul(
                                            pa[:], tcur[:, hh, jj, :, fsl],
                                            att1[(a, hh, jj)][:],
                                            start=(k == 0), stop=(k == 3),
                                            perf_mode=DR)
                                        k += 1
                                sc = cvp[:, fc2, s, a:a + 1]
                                if a == 0:
                                    nc.vector.tensor_scalar_mul(
                                        acc[:], pa[:], sc)
                                elif s < P - 1 and a == A - 1:
                                    nc.vector.scalar_tensor_tensor(
                                        a8[:], pa[:], sc, acc[:],
                                        op0=ALU.mult, op1=ALU.add)
                                else:
                                    nc.vector.scalar_tensor_tensor(
                                        acc[:], pa[:], sc, acc[:],
                                        op0=ALU.mult, op1=ALU.add)
                            if s < P - 1:
                                for qs in range(2):
                                    pt8 = tp8.tile([128, 128], BF16, tag="tp8")
                                    nc.tensor.transpose(
                                        pt8[:], a8[:, qs * 128:(qs + 1) * 128],
                                        ident8[:])
                                    nc.vector.tensor_copy(
                                        accr8[:, qs, fsl], pt8[:])
                            else:
                                for qs in range(2):
                                    ptr = tp.tile([128, 128], F32R, tag="tp")
                                    nc.tensor.transpose(
                                        ptr[:], acc[:, qs * 128:(qs + 1) * 128],
                                        ident[:])
                                    nc.vector.tensor_scalar_mul(
                                        attn[:, qs, fsl], ptr[:],
                                        invd[:, qs:qs + 1])
                    else:
                        for qs in range(2):
                            pas = []
                            for a in range(A):
                                pa = ps.tile([128, D], F32, tag="pa")
                                if mode == "f2f8":
                                    k = 0
                                    for h in range(2):
                                        for j in range(2):
                                            for pl in range(2):
                                                nc.tensor.matmul(
                                                    pa[:],
                                                    at[(a, h, j)][:, pl,
                                                        qs * 128:(qs + 1) * 128],
                                                    tcur[:, h, j, pl, :],
                                                    start=(k == 0), stop=(k == 7))
                                                k += 1
                                else:
                                    k = 0
                                    for h in range(2):
                                        for j in range(2):
                                            nc.tensor.matmul(
                                                pa[:],
                                                at[(a, h, j)][:, :,
                                                    qs * 128:(qs + 1) * 128],
                                                tcur[:, h, j, :, :],
                                                start=(k == 0), stop=(k == 3),
                                                perf_mode=DR)
                                            k += 1
                                pas.append(pa)
                            if s < P - 1:
                                combine(s, qs, pas, accr8[:, qs, :])
                            else:
                                accf = wrk.tile([128, D], F32, tag="accf")
                                combine(s, qs, pas, accf[:])
                                nc.vector.tensor_scalar_mul(
                                    attn[:, qs, :], accf[:], invd[:, qs:qs + 1])

                    if s < P - 1:
                        exin = dram.tile([128, 2, D], F8, tag=f"exi{rep}_{s}")
                        exout = dram.tile([4, 128, 2, D], F8, tag=f"exo{rep}_{s}")
                        nc.sync.dma_start(exin[:], accr8[:])
                        if no_coll:
                            for g_ in range(4):
                                nc.sync.dma_start(exout[g_], exin[:])
                        else:
                            nc.gpsimd.collective_compute(
                                "AllGather", ALU.bypass, replica_groups=ag4,
                                ins=[exin.opt()], outs=[exout.opt()])
                        tnext = tt.tile([128, 2, 2, 2, D], F8, tag="T")
                        for g in range(4):
                            nc.sync.dma_start(
                                tnext[:, :, g >> 1, g & 1, :], exout[g])
                        tcur = tnext
                assert attn is not None

                # ---- tail for our 256-row n-quarter (2 blocks of 128) ----
                if no_tail:
                    zot = wrk.tile([128, D], F32, tag="ot")
                    nc.vector.tensor_copy(zot[:], attn[:, 0, :])
                    for i in range(2):
                        nc.sync.dma_start(out_p[i], zot[:])
                    continue

                # attn^T (4 transposes) for the W0 contraction over f
                attnT = wrk.tile([128, 2, D], F32R, tag="attnT")
                for i in range(2):
                    for dc in range(2):
                        ptr = tp.tile([128, 128], F32R, tag="tp")
                        nc.tensor.transpose(
                            ptr[:], attn[:, i, dc * 128:(dc + 1) * 128], ident[:])
                        nc.vector.tensor_copy(
                            attnT[:, dc, i * 128:(i + 1) * 128], ptr[:])

                resid = wrk.tile([128, 2, D], F32, tag="resid")
                prs = []
                for i in range(2):
                    pr = ps.tile([128, D], F32, tag="pa")
                    for dc in range(2):
                        nc.tensor.matmul(
                            pr[:], attnT[:, dc, i * 128:(i + 1) * 128],
                            w0[:, dc, :], start=(dc == 0), stop=(dc == 1))
                    prs.append(pr)
                for i in range(2):
                    nc.vector.tensor_add(resid[:, i, :], prs[i][:], xtl[:, i, :])

                # LN1: h = (resid - m)/sqrt(v+eps) * gamma2 + beta2
                hn = wrk.tile([128, 2, D], F32, tag="hn")
                for i in range(2):
                    st = wrk.tile([128, 6], F32, tag=f"st{i}")
                    mv = wrk.tile([128, 2], F32, tag=f"mv{i}")
                    nc.vector.bn_stats(st[:], resid[:, i, :])
                    nc.vector.bn_aggr(mv[:], st[:])
                    rstd = wrk.tile([128, 1], F32, tag=f"rstd{i}")
                    nc.scalar.activation(rstd[:], mv[:, 1:2], ACTF.Sqrt,
                                         bias=epst[:], scale=1.0)
                    nc.vector.reciprocal(rstd[:], rstd[:])
                    nc.vector.tensor_scalar(hn[:, i, :], resid[:, i, :],
                                            mv[:, 0:1], rstd[:],
                                            op0=ALU.subtract, op1=ALU.mult)
                hb = wrk.tile([128, 2, D], F32R, tag="hb")
                hnf = hn[:].rearrange("p a b -> p (a b)")
                nc.vector.tensor_mul(hnf, hnf,
                                     g2b[:].rearrange("p a b -> p (a b)"))
                nc.vector.tensor_add(hb[:].rearrange("p a b -> p (a b)"), hnf,
                                     b2b[:].rearrange("p a b -> p (a b)"))

                # h^T, then f = gelu(h @ W1 + b1)
                ht = wrk.tile([128, 2, D], F32R, tag="ht")
                for i in range(2):
                    for dc in range(2):
                        ptr = tp.tile([128, 128], F32R, tag="tp")
                        nc.tensor.transpose(
                            ptr[:], hb[:, i, dc * 128:(dc + 1) * 128], ident[:])
                        nc.vector.tensor_copy(
                            ht[:, dc, i * 128:(i + 1) * 128], ptr[:])
                g1 = wrk.tile([128, 2, 2 * D], F32, tag="g1")
                pf = psf.tile([128, 2, 2 * D], F32, tag="pf")
                for i in range(2):
                    for dc in range(2):
                        nc.tensor.matmul(
                            pf[:, i, :], ht[:, dc, i * 128:(i + 1) * 128],
                            w1[:, dc, :], start=(dc == 0), stop=(dc == 1))
                f1 = wrk.tile([128, 2, 2 * D], F32, tag="f1")
                nc.vector.tensor_add(f1[:].rearrange("p a b -> p (a b)"),
                                     pf[:].rearrange("p a b -> p (a b)"),
                                     b1b[:].rearrange("p a b -> p (a b)"))
                nc.scalar.activation(g1[:].rearrange("p a b -> p (a b)"),
                                     f1[:].rearrange("p a b -> p (a b)"),
                                     ACTF.Gelu)

                # LN2 * gf + bf
                fn = wrk.tile([128, 2, 2 * D], F32, tag="fn")
                for i in range(2):
                    st2 = wrk.tile([128, 6], F32, tag=f"st2_{i}")
                    mv2 = wrk.tile([128, 2], F32, tag=f"mv2_{i}")
                    nc.vector.bn_stats(st2[:], g1[:, i, :])
                    nc.vector.bn_aggr(mv2[:], st2[:])
                    rstd2 = wrk.tile([128, 1], F32, tag=f"rstd2_{i}")
                    nc.scalar.activation(rstd2[:], mv2[:, 1:2], ACTF.Sqrt,
                                         bias=epst[:], scale=1.0)
                    nc.vector.reciprocal(rstd2[:], rstd2[:])
                    nc.vector.tensor_scalar(fn[:, i, :], g1[:, i, :],
                                            mv2[:, 0:1], rstd2[:],
                                            op0=ALU.subtract, op1=ALU.mult)
                f2 = wrk.tile([128, 2, 2 * D], F32R, tag="f2")
                fnf = fn[:].rearrange("p a b -> p (a b)")
                nc.vector.tensor_mul(fnf, fnf,
                                     gfb[:].rearrange("p a b -> p (a b)"))
                nc.vector.tensor_add(f2[:].rearrange("p a b -> p (a b)"), fnf,
                                     bfb[:].rearrange("p a b -> p (a b)"))

                # f2^T, then out = f2 @ W2 + b2f + resid
                f2t = wrk.tile([128, 4, D], F32R, tag="f2t")
                for i in range(2):
                    for k in range(4):
                        ptr = tp.tile([128, 128], F32R, tag="tp")
                        nc.tensor.transpose(
                            ptr[:], f2[:, i, k * 128:(k + 1) * 128], ident[:])
                        nc.vector.tensor_copy(
                            f2t[:, k, i * 128:(i + 1) * 128], ptr[:])
                for i in range(2):
                    po = ps.tile([128, D], F32, tag="pa")
                    for k in range(4):
                        nc.tensor.matmul(po[:], f2t[:, k, i * 128:(i + 1) * 128],
                                         w2[:, k, :], start=(k == 0),
                                         stop=(k == 3))
                    ot = wrk.tile([128, D], F32, tag=f"ot{i}")
                    nc.vector.tensor_add(ot[:], po[:], b2fb[:])
                    nc.vector.tensor_add(ot[:], ot[:], resid[:, i, :])
                    nc.sync.dma_start(out_p[i], ot[:])

    nc.finalize()
    return nc


def _build_nc_v2(null: bool = False, reps: int = 1, triv: bool = True,
                 no_tail: bool = False, no_coll: bool = False,
                 mode: str = "v2", tt_bufs: int = 3,
                 exchange: str | None = None):
    """All-quarter chain with direct SBUF remote-DMA exchanges.

    Sharding: core c -> (b = c>>2, q = c&3).  Every chain step computes only
    the core's 256-row n-quarter; full-n T is restored after steps 0-2 by a
    3-way peer broadcast (remote_dma_broadcast, XOR-relative routing: slot k
    holds quarter q^k; the per-core host-side chunk permutation makes the
    layout consistent -- the trn2 die-flip x^2 on the upper quad cancels
    under XOR addressing).  No collective firmware, no HBM bounce.

    V projection is fp8 DoubleRow from pre-scaled x/Wv (XSC/WSC); per-core
    input drops to ~3.3MB (vs 15.7MB for f1drh).  triv=True folds away the
    zero/one bias and LN vectors of setup_inputs (checked at runtime in
    kernel()).
    """
    if exchange is None:
        exchange = EXCHANGE
    if no_coll:
        exchange = "none"
    nc = bacc.Bacc("TRN2", target_bir_lowering=False, debug=False,
                   num_devices=NCORES)
    ag4 = [[4 * g + i for i in range(4)] for g in range(NCORES // 4)]

    dp = nc.declare_dram_parameter
    atq_in = dp("atq8", [A, 4, 128, 2, NQ], F8, isOutput=False)
    xt8_in = dp("xt8", [128, 2, N], F8, isOutput=False)      # (x*XSC)^T, perm
    wv8_in = dp("wv8", [128, 2, D], F8, isOutput=False)      # Wv_cat*WSC
    if not triv:
        bv_in = dp("bv", [MC, 128, D], F32, isOutput=False)  # perm, * S0
    xtail_in = dp("xtail", [2, 128, D], F32, isOutput=False)
    w0_in = dp("w0", [2, 128, D], F32R, isOutput=False)
    w1_in = dp("w1", [2, 128, 2 * D], F32R, isOutput=False)
    w2_in = dp("w2", [4, 128, D], F32R, isOutput=False)
    cvp_in = dp("cvp", [128, 2, P, A], F32, isOutput=False)
    invd_in = dp("invd", [128, 2], F32, isOutput=False)
    id_in = dp("ident", [128, 128], F32R, isOutput=False)
    id8_in = dp("ident8", [128, 128], BF16, isOutput=False)
    if not triv:
        g2_in = dp("g2", [128, 2, D], F32, isOutput=False)
        b2_in = dp("b2", [128, 2, D], F32, isOutput=False)
        gf_in = dp("gf", [128, 2, 2 * D], F32, isOutput=False)
        bf_in = dp("bf", [128, 2, 2 * D], F32, isOutput=False)
        b1_in = dp("b1", [128, 2, 2 * D], F32, isOutput=False)
        b2f_in = dp("b2f", [128, D], F32, isOutput=False)
    out_p = dp("out", [2, 128, D], F32, isOutput=True)

    with tile.TileContext(nc) as tc:
        with (
            tc.tile_pool(name="cst", bufs=1) as cst,
            tc.tile_pool(name="wrk", bufs=1) as wrk,
            tc.tile_pool(name="bvp", bufs=2) as bvp,
            tc.tile_pool(name="tt", bufs=tt_bufs) as tt,
            tc.tile_pool(name="ps", bufs=3, space="PSUM") as ps,
            tc.tile_pool(name="psf", bufs=1, space="PSUM") as psf,
            tc.tile_pool(name="tp", bufs=2, space="PSUM") as tp,
            tc.tile_pool(name="tp8", bufs=1, space="PSUM") as tp8,
            tc.tile_pool(name="dram", bufs=1, space="DRAM") as dram,
        ):
            # Warm the ACT engine before anything depends on it: the dummy
            # Sqrt pulls in the sqrt-containing act table during the input
            # DMA wait (Copy is in every set, so the chain's ACT copies keep
            # it loaded and LN1's Sqrt needs no table swap).
            epst = cst.tile([128, 1], F32)
            nc.vector.memset(epst[:], EPS)
            actwarm = cst.tile([128, 1], F32)
            nc.scalar.activation(actwarm[:], epst[:], ACTF.Sqrt)

            # V-projection-critical inputs first, then per-relation adjacency
            # chunks (step 0's a-loop starts after the first 256KB lands),
            # then the tail weights (needed ~15us later).
            xt8 = cst.tile([128, 2, N], F8)
            nc.sync.dma_start(xt8[:], xt8_in[:])
            wv8 = cst.tile([128, 2, D], F8)
            nc.sync.dma_start(wv8[:], wv8_in[:])
            cvp = cst.tile([128, 2, P, A], F32)
            nc.sync.dma_start(cvp[:], cvp_in[:])
            invd = cst.tile([128, 2], F32)
            nc.sync.dma_start(invd[:], invd_in[:])
            ident8 = cst.tile([128, 128], BF16)
            nc.sync.dma_start(ident8[:], id8_in[:])
            atq = cst.tile([128, A, 4, 2, NQ], F8)
            for a_ in range(A):
                nc.sync.dma_start(atq[:, a_],
                                  atq_in[a_].rearrange("k p pl q -> p k pl q"))
            ident = cst.tile([128, 128], F32R)
            nc.sync.dma_start(ident[:], id_in[:])
            w0 = cst.tile([128, 2, D], F32R)
            nc.sync.dma_start(w0[:], w0_in.rearrange("c p f -> p c f"))
            w1 = cst.tile([128, 2, 2 * D], F32R)
            nc.sync.dma_start(w1[:], w1_in.rearrange("c p f -> p c f"))
            w2 = cst.tile([128, 4, D], F32R)
            nc.sync.dma_start(w2[:], w2_in.rearrange("c p f -> p c f"))
            xtl = cst.tile([128, 2, D], F32)
            nc.sync.dma_start(xtl[:], xtail_in.rearrange("c p f -> p c f"))
            epst = cst.tile([128, 1], F32)
            nc.vector.memset(epst[:], EPS)
            if not triv:
                g2b = cst.tile([128, 2, D], F32)
                nc.sync.dma_start(g2b[:], g2_in[:])
                b2b = cst.tile([128, 2, D], F32)
                nc.sync.dma_start(b2b[:], b2_in[:])
                gfb = cst.tile([128, 2, 2 * D], F32)
                nc.sync.dma_start(gfb[:], gf_in[:])
                bfb = cst.tile([128, 2, 2 * D], F32)
                nc.sync.dma_start(bfb[:], bf_in[:])
                b1b = cst.tile([128, 2, 2 * D], F32)
                nc.sync.dma_start(b1b[:], b1_in[:])
                b2fb = cst.tile([128, D], F32)
                nc.sync.dma_start(b2fb[:], b2f_in[:])

            # one receive sem per chain step: a fast peer's step-(s+1)
            # arrivals must not count toward our step-s threshold.  Reuse
            # across reps is safe: our step-s wait precedes our step-(s+1)
            # sends, which gate every peer's next-rep step-s send.
            rsems = [nc.alloc_semaphore(f"rsem{s}") for s in range(P - 1)]
            lsem = nc.alloc_semaphore("lsem")

            if null:
                zo = wrk.tile([128, D], F32, tag="ot")
                nc.vector.memset(zo[:], 0.0)
                for i in range(2):
                    nc.sync.dma_start(out_p[i], zo[:])

            for rep in range(0 if null else reps):
                # ---- T0 = fp8(V * S0): one DR pass per 128-row chunk ----
                if rep and exchange == "rdma":
                    # T tiles reuse prior-rep buffers; ensure the previous
                    # rep's send-DMA reads of slot 0 retired (3 exch * 48).
                    # On DVE: in-order execution gates the V writes below.
                    with tc.tile_critical():
                        nc.vector.wait_ge(lsem, 144 * rep)
                tcur = tt.tile([128, 4, 2, D], F8, tag="T")
                for m in range(MC):
                    pv = ps.tile([128, D], F32, tag="pa")
                    nc.tensor.matmul(pv[:], xt8[:, :, m * 128:(m + 1) * 128],
                                     wv8[:], start=True, stop=True,
                                     perf_mode=DR)
                    dst = tcur[:, m >> 1, m & 1, :]
                    if triv:
                        nc.scalar.activation(dst, pv[:], ACTF.Copy,
                                             scale=SCALES[0] / (XSC * WSC))
                    else:
                        bvt = bvp.tile([128, D], F32, tag="bv")
                        nc.sync.dma_start(bvt[:], bv_in[m])
                        nc.vector.scalar_tensor_tensor(
                            dst, pv[:], SCALES[0] / (XSC * WSC), bvt[:],
                            op0=ALU.mult, op1=ALU.add)

                # ---- chain: 4 quarter steps; exchange T after steps 0-2 ----
                attn = None
                for s in range(P):
                    last = s == P - 1
                    tnext = None
                    accr8 = None
                    if not last:
                        tnext = tt.tile([128, 4, 2, D], F8, tag="T")
                        if exchange == "coll":
                            accr8 = wrk.tile([128, 2, D], F8, tag="accr8")
                    else:
                        attn = wrk.tile([128, 2, D], F32R, tag="attn")
                    for fc2 in range(2):
                        fsl = slice(fc2 * 128, (fc2 + 1) * 128)
                        acc = wrk.tile([128, NQ], F32R if last else BF16,
                                       tag="accL" if last else "acc8")
                        for a in range(A):
                            pa = ps.tile([128, NQ], F32, tag="pa")
                            for k in range(4):
                                nc.tensor.matmul(
                                    pa[:], tcur[:, k, :, fsl], atq[:, a, k],
                                    start=(k == 0), stop=(k == 3),
                                    perf_mode=DR)
                            sc = cvp[:, fc2, s, a:a + 1]
                            if a == 0:
                                nc.vector.tensor_scalar_mul(acc[:], pa[:], sc)
                            else:
                                nc.vector.scalar_tensor_tensor(
                                    acc[:], pa[:], sc, acc[:],
                                    op0=ALU.mult, op1=ALU.add)
                        for qs2 in range(2):
                            qsl = slice(qs2 * 128, (qs2 + 1) * 128)
                            if not last:
                                pt8 = tp8.tile([128, 128], BF16, tag="tp8")
                                nc.tensor.transpose(pt8[:], acc[:, qsl],
                                                    ident8[:])
                                dst8 = (accr8[:, qs2, fsl]
                                        if exchange == "coll"
                                        else tnext[:, 0, qs2, fsl])
                                nc.scalar.activation(dst8, pt8[:], ACTF.Copy)
                            else:
                                ptr = tp.tile([128, 128], F32R, tag="tp")
                                nc.tensor.transpose(ptr[:], acc[:, qsl],
                                                    ident[:])
                                nc.scalar.activation(
                                    attn[:, qs2, fsl], ptr[:], ACTF.Copy,
                                    scale=invd[:, qs2:qs2 + 1])
                    if not last:
                        if exchange == "coll":
                            exin = dram.tile([128, 2, D], F8,
                                             tag=f"exi{rep}_{s}")
                            exout = dram.tile([4, 128, 2, D], F8,
                                              tag=f"exo{rep}_{s}")
                            nc.sync.dma_start(exin[:], accr8[:])
                            nc.gpsimd.collective_compute(
                                "AllGather", ALU.bypass, replica_groups=ag4,
                                ins=[exin.opt()], outs=[exout.opt()])
                            nc.sync.dma_start(
                                tnext[:], exout.rearrange("g p c f -> p g c f"))
                        elif exchange == "none":
                            for dlt in (1, 2, 3):
                                nc.sync.dma_start(tnext[:, dlt, :, :],
                                                  tnext[:, 0, :, :])
                        else:
                            with tc.tile_critical():
                                for dlt in (1, 2, 3):
                                    rd = [None] * 8
                                    rd[dlt] = (0, dlt)
                                    nc.gpsimd.remote_dma_broadcast(
                                        tnext[:, dlt, :, :],
                                        tnext[:, 0, :, :],
                                        rsems[s], lsem, rdests=rd)
                                nc.gpsimd.trigger_dma(count=None)
                                nc.gpsimd.wait_ge(rsems[s], 6 * (rep + 1))
                        tcur = tnext
                assert attn is not None

                if no_tail:
                    zot = wrk.tile([128, D], F32, tag="ot")
                    nc.vector.tensor_copy(zot[:], attn[:, 0, :])
                    for i in range(2):
                        nc.sync.dma_start(out_p[i], zot[:])
                    continue

                # ---- tail: W0 + residual, LN, FFN (as f1drh) ----
                attnT = wrk.tile([128, 2, D], F32R, tag="attnT")
                for i in range(2):
                    for dc in range(2):
                        ptr = tp.tile([128, 128], F32R, tag="tp")
                        nc.tensor.transpose(
                            ptr[:], attn[:, i, dc * 128:(dc + 1) * 128],
                            ident[:])
                        nc.vector.tensor_copy(
                            attnT[:, dc, i * 128:(i + 1) * 128], ptr[:])

                resid = wrk.tile([128, 2, D], F32, tag="resid")
                prs = []
                for i in range(2):
                    pr = ps.tile([128, D], F32, tag="pa")
                    for dc in range(2):
                        nc.tensor.matmul(
                            pr[:], attnT[:, dc, i * 128:(i + 1) * 128],
                            w0[:, dc, :], start=(dc == 0), stop=(dc == 1))
                    prs.append(pr)
                for i in range(2):
                    nc.vector.tensor_add(resid[:, i, :], prs[i][:],
                                         xtl[:, i, :])

                # LN1 (triv: gamma=1, beta=0 -> normalized value directly)
                hb = wrk.tile([128, 2, D], F32R, tag="hb")
                for i in range(2):
                    st = wrk.tile([128, 6], F32, tag=f"st{i}")
                    mv = wrk.tile([128, 2], F32, tag=f"mv{i}")
                    nc.vector.bn_stats(st[:], resid[:, i, :])
                    nc.vector.bn_aggr(mv[:], st[:])
                    rstd = wrk.tile([128, 1], F32, tag=f"rstd{i}")
                    nc.scalar.activation(rstd[:], mv[:, 1:2], ACTF.Sqrt,
                                         bias=epst[:], scale=1.0)
                    nc.vector.reciprocal(rstd[:], rstd[:])
                    nc.vector.tensor_scalar(hb[:, i, :], resid[:, i, :],
                                            mv[:, 0:1], rstd[:],
                                            op0=ALU.subtract, op1=ALU.mult)
                if not triv:
                    hbf = hb[:].rearrange("p a b -> p (a b)")
                    nc.vector.tensor_mul(hbf, hbf,
                                         g2b[:].rearrange("p a b -> p (a b)"))
                    nc.vector.tensor_add(hbf, hbf,
                                         b2b[:].rearrange("p a b -> p (a b)"))

                ht = wrk.tile([128, 2, D], F32R, tag="ht")
                for i in range(2):
                    for dc in range(2):
                        ptr = tp.tile([128, 128], F32R, tag="tp")
                        nc.tensor.transpose(
                            ptr[:], hb[:, i, dc * 128:(dc + 1) * 128],
                            ident[:])
                        nc.vector.tensor_copy(
                            ht[:, dc, i * 128:(i + 1) * 128], ptr[:])
                g1 = wrk.tile([128, 2, 2 * D], F32, tag="g1")
                pf = psf.tile([128, 2, 2 * D], F32, tag="pf")
                for i in range(2):
                    for dc in range(2):
                        nc.tensor.matmul(
                            pf[:, i, :], ht[:, dc, i * 128:(i + 1) * 128],
                            w1[:, dc, :], start=(dc == 0), stop=(dc == 1))
                if triv:
                    nc.scalar.activation(g1[:].rearrange("p a b -> p (a b)"),
                                         pf[:].rearrange("p a b -> p (a b)"),
                                         ACTF.Gelu)
                else:
                    f1t = wrk.tile([128, 2, 2 * D], F32, tag="f1")
                    nc.vector.tensor_add(f1t[:].rearrange("p a b -> p (a b)"),
                                         pf[:].rearrange("p a b -> p (a b)"),
                                         b1b[:].rearrange("p a b -> p (a b)"))
                    nc.scalar.activation(g1[:].rearrange("p a b -> p (a b)"),
                                         f1t[:].rearrange("p a b -> p (a b)"),
                                         ACTF.Gelu)

                # LN2 (triv: gf=1, bf=0)
                f2 = wrk.tile([128, 2, 2 * D], F32R, tag="f2")
                for i in range(2):
                    st2 = wrk.tile([128, 6], F32, tag=f"st2_{i}")
                    mv2 = wrk.tile([128, 2], F32, tag=f"mv2_{i}")
                    nc.vector.bn_stats(st2[:], g1[:, i, :])
                    nc.vector.bn_aggr(mv2[:], st2[:])
                    rstd2 = wrk.tile([128, 1], F32, tag=f"rstd2_{i}")
                    nc.scalar.activation(rstd2[:], mv2[:, 1:2], ACTF.Sqrt,
                                         bias=epst[:], scale=1.0)
                    nc.vector.reciprocal(rstd2[:], rstd2[:])
                    nc.vector.tensor_scalar(f2[:, i, :], g1[:, i, :],
                                            mv2[:, 0:1], rstd2[:],
                                            op0=ALU.subtract, op1=ALU.mult)
                if not triv:
                    f2f = f2[:].rearrange("p a b -> p (a b)")
                    nc.vector.tensor_mul(f2f, f2f,
                                         gfb[:].rearrange("p a b -> p (a b)"))
                    nc.vector.tensor_add(f2f, f2f,
                                         bfb[:].rearrange("p a b -> p (a b)"))

                f2t = wrk.tile([128, 4, D], F32R, tag="f2t")
                for i in range(2):
                    for k in range(4):
                        ptr = tp.tile([128, 128], F32R, tag="tp")
                        nc.tensor.transpose(
                            ptr[:], f2[:, i, k * 128:(k + 1) * 128], ident[:])
                        nc.vector.tensor_copy(
                            f2t[:, k, i * 128:(i + 1) * 128], ptr[:])
                for i in range(2):
                    po = ps.tile([128, D], F32, tag="pa")
                    for k in range(4):
                        nc.tensor.matmul(po[:],
                                         f2t[:, k, i * 128:(i + 1) * 128],
                                         w2[:, k, :], start=(k == 0),
                                         stop=(k == 3))
                    ot = wrk.tile([128, D], F32, tag=f"ot{i}")
                    if triv:
                        nc.vector.tensor_add(ot[:], po[:], resid[:, i, :])
                    else:
                        nc.vector.tensor_add(ot[:], po[:], b2fb[:])
                        nc.vector.tensor_add(ot[:], ot[:], resid[:, i, :])
                    nc.sync.dma_start(out_p[i], ot[:])

    nc.finalize()
    return nc


def _prep_in_maps_v2(adjacency, degree, x, kernels, Wv, Bv, W0, gamma2, beta2,
                     W1, b1, gf, bf, W2, b2f):
    f8np = mybir.dt.np(F8)
    soft = _softmax_relu(np.asarray(kernels, np.float32))
    wv_cat = np.ascontiguousarray(
        np.transpose(np.asarray(Wv, np.float32), (1, 0, 2)).reshape(D, D))
    bv_cat = np.ascontiguousarray(
        np.transpose(np.asarray(Bv, np.float32), (1, 0, 2)).reshape(N, D))
    invd_full = 1.0 / np.diagonal(np.asarray(degree, np.float32),
                                  axis1=1, axis2=2)
    eye = np.eye(128, dtype=np.float32)
    eye8 = np.eye(128).astype(mybir.dt.np(BF16))
    ones128 = np.ones((128, 1), np.float32)

    def dup(v):
        t = ones128 * np.asarray(v, np.float32)[None, :]
        return np.ascontiguousarray(np.stack([t, t], axis=1))

    g2 = dup(gamma2)
    b2 = dup(beta2)
    gfB = dup(gf)
    bfB = dup(bf)
    b1B = dup(b1)
    b2fB = ones128 * np.asarray(b2f, np.float32)[None, :]
    w0r = np.ascontiguousarray(np.asarray(W0, np.float32).reshape(2, 128, D))
    w1r = np.ascontiguousarray(
        np.asarray(W1, np.float32).reshape(2, 128, 2 * D))
    w2r = np.ascontiguousarray(np.asarray(W2, np.float32).reshape(4, 128, D))

    cvec = np.empty((P, A, H), np.float32)
    for s in range(P):
        fac = (SCALES[s + 1] / (512.0 * SCALES[s]) if s < P - 1
               else 1.0 / (512.0 * SCALES[P - 1]))
        cvec[s] = soft[:, :, P - 1 - s].T * fac
    hidx = np.arange(D) // DH
    cvpart = np.empty((128, 2, P, A), np.float32)
    for fc2 in range(2):
        cvpart[:, fc2, :, :] = cvec[
            :, :, hidx[fc2 * 128:(fc2 + 1) * 128]].transpose(2, 0, 1)

    wv8 = np.ascontiguousarray(
        (wv_cat * WSC).reshape(2, 128, D).transpose(1, 0, 2)).astype(f8np)

    adjacency = np.asarray(adjacency, np.float32)
    x = np.asarray(x, np.float32)

    in_maps = []
    for c in range(NCORES):
        b = c >> 2
        q = c & 3
        nsl = slice(q * NQ, (q + 1) * NQ)
        # chunk m lives at (slot k, plane pl); rdma: slot k = quarter q^k
        # (XOR-relative routing), coll/none: slot k = quarter k (AllGather
        # rank order)
        if EXCHANGE == "rdma":
            mlist = [2 * (q ^ k) + pl for k in range(4) for pl in range(2)]
        else:
            mlist = [2 * k + pl for k in range(4) for pl in range(2)]

        # A^T n-rows (contraction) in XOR order; cols = own quarter
        aq = adjacency[b].transpose(0, 2, 1)[:, :, nsl]       # [A, m, 256]
        a8 = (512.0 * aq).reshape(A, MC, 128, NQ)[:, mlist]
        atq8 = np.ascontiguousarray(
            a8.reshape(A, 4, 2, 128, NQ).transpose(0, 1, 3, 2, 4)
        ).astype(f8np)                                        # [A,k,128,pl,q]

        # x^T fp8, column blocks in XOR chunk order
        xp = (XSC * x[b][np.concatenate(
            [np.arange(128 * m_, 128 * m_ + 128) for m_ in mlist])])
        xt8 = np.ascontiguousarray(
            xp.T.reshape(2, 128, N).transpose(1, 0, 2)).astype(f8np)

        bvp = np.ascontiguousarray(
            (bv_cat * SCALES[0]).reshape(MC, 128, D)[mlist])

        xtail_c = np.ascontiguousarray(x[b, nsl]).reshape(2, 128, D)
        invd_c = np.ascontiguousarray(invd_full[b][nsl].reshape(2, 128).T)

        in_maps.append({
            "atq8": atq8,
            "xt8": xt8,
            "wv8": wv8,
            "bv": bvp,
            "xtail": xtail_c,
            "w0": w0r, "w1": w1r, "w2": w2r,
            "cvp": cvpart,
            "invd": invd_c,
            "ident": eye,
            "ident8": eye8,
            "g2": g2, "b2": b2, "gf": gfB, "bf": bfB, "b1": b1B, "b2f": b2fB,
        })
    return in_maps


def _softmax_relu(kernels):
    r = np.maximum(kernels, 0.0)
    e = np.exp(r - r.max(axis=1, keepdims=True))
    return (e / e.sum(axis=1, keepdims=True)).astype(np.float32)  # [H, A, P]


def _prep_in_maps(adjacency, degree, x, kernels, Wv, Bv, W0, gamma2, beta2,
                  W1, b1, gf, bf, W2, b2f, full_at=None):
    if MODE == "v2":
        return _prep_in_maps_v2(adjacency, degree, x, kernels, Wv, Bv, W0,
                                gamma2, beta2, W1, b1, gf, bf, W2, b2f)
    f8np = mybir.dt.np(F8)
    atfull_cache = {}
    if full_at is None:
        full_at = MODE in ("f1drx", "f1drh", "f1drh2")
    soft = _softmax_relu(np.asarray(kernels, np.float32))
    wv_cat = np.ascontiguousarray(
        np.transpose(np.asarray(Wv, np.float32), (1, 0, 2)).reshape(D, D))
    bv_cat = np.ascontiguousarray(
        np.transpose(np.asarray(Bv, np.float32), (1, 0, 2)).reshape(N, D))
    invd_full = 1.0 / np.diagonal(np.asarray(degree, np.float32),
                                  axis1=1, axis2=2)  # [B, N]
    eye = np.eye(128, dtype=np.float32)
    ones128 = np.ones((128, 1), np.float32)

    def dup(v):
        t = ones128 * np.asarray(v, np.float32)[None, :]
        return np.ascontiguousarray(np.stack([t, t], axis=1))
    g2 = dup(gamma2)
    b2 = dup(beta2)
    gfB = dup(gf)
    bfB = dup(bf)
    b1B = dup(b1)
    b2fB = ones128 * np.asarray(b2f, np.float32)[None, :]
    w0r = np.ascontiguousarray(np.asarray(W0, np.float32).reshape(2, 128, D))
    w1r = np.ascontiguousarray(np.asarray(W1, np.float32).reshape(2, 128, 2 * D))
    w2r = np.ascontiguousarray(np.asarray(W2, np.float32).reshape(4, 128, D))

    # combine coefficients: chain step s applies mix P-1-s; fold in the fp8
    # stage scales (adjacency pre-scaled by 512, T-hat_s = T_s * S[s]).
    cvec = np.empty((128, P, A, H), np.float32)
    for s in range(P):
        fac = (SCALES[s + 1] / (512.0 * SCALES[s]) if s < P - 1
               else 1.0 / (512.0 * SCALES[P - 1]))
        cvec[:, s, :, :] = (soft[:, :, P - 1 - s].T * fac)[None, :, :]
    # flat [P*A, 128, D] f-column-expanded coef (mode f2drf)
    cvflat = np.ascontiguousarray(
        np.repeat(cvec[0].reshape(P * A, 1, H), DH, axis=2)
        .reshape(P * A, 1, D) * np.ones((1, 128, 1), np.float32))
    # per-f-row coef [128, 2, P, A] (mode f1dr)
    hidx = np.arange(D) // DH
    cvpart = np.empty((128, 2, P, A), np.float32)
    for fc2 in range(2):
        cvpart[:, fc2, :, :] = cvec[0][
            :, :, hidx[fc2 * 128:(fc2 + 1) * 128]].transpose(2, 0, 1)
    eye8 = np.eye(128).astype(mybir.dt.np(BF16))

    adjacency = np.asarray(adjacency, np.float32)
    x = np.asarray(x, np.float32)
    mperm = [0, 2, 4, 6, 1, 3, 5, 7]  # m-chunk order for (h, j, plane)

    in_maps = []
    for c in range(NCORES):
        b = c >> 2
        q = c & 3
        nsl = slice(q * NQ, (q + 1) * NQ)

        def at_pack(cols):
            w = cols.stop - cols.start
            aq = adjacency[b].transpose(0, 2, 1)[:, :, cols]  # [A, n, q]
            a8 = (512.0 * aq).reshape(A, MC, 128, w)[:, mperm]
            return np.ascontiguousarray(
                a8.reshape(A, 2, 2, 2, 128, w).transpose(0, 1, 2, 4, 3, 5)
            ).astype(f8np)                                 # [A,h,j,128,plane,q]
        if full_at:
            at8 = atfull_cache.get(b)
            if at8 is None:
                at8 = atfull_cache[b] = at_pack(slice(0, N))
            atq8 = at_pack(nsl)
        else:
            at8 = at_pack(nsl)
            atq8 = at8
        xt_c = np.ascontiguousarray(x[b].T).reshape(2, 128, N)
        xtail_c = np.ascontiguousarray(x[b, nsl]).reshape(2, 128, D)
        invd_c = np.ascontiguousarray(invd_full[b][nsl].reshape(2, 128).T)

        in_maps.append({
            "at8": at8,
            "atq8": atq8,
            "xt": xt_c,
            "xtail": xtail_c,
            "wv": wv_cat.reshape(2, 128, D),
            "bv": (bv_cat * SCALES[0]).reshape(MC, 128, D),
            "w0": w0r, "w1": w1r, "w2": w2r,
            "cv": cvec,
            "cvf": cvflat,
            "cvp": cvpart,
            "ident8": eye8,
            "invd": invd_c,
            "g2": g2, "b2": b2, "gf": gfB, "bf": bfB, "b1": b1B, "b2f": b2fB,
            "ident": eye,
        })
    return in_maps


def _inputs_trivial(inputs) -> bool:
    """True when the bias/LN vectors match setup_inputs' constants."""
    z = lambda a: not np.any(np.asarray(a))
    o = lambda a: np.all(np.asarray(a) == 1.0)
    return (z(inputs["Bv"]) and o(inputs["gamma2"]) and z(inputs["beta2"])
            and z(inputs["b1"]) and o(inputs["gf"]) and z(inputs["bf"])
            and z(inputs["b2f"]))


def kernel(**inputs) -> np.ndarray:
    if MODE == "v2":
        triv = _inputs_trivial(inputs)
        key = ("v2", EXCHANGE, triv)
        if key not in _CACHE:
            _CACHE[key] = _build_nc_v2(triv=triv)
        nc = _CACHE[key]
    else:
        if "nc" not in _CACHE:
            _CACHE["nc"] = _build_nc(mode=MODE)
        nc = _CACHE["nc"]
    in_maps = _prep_in_maps(**inputs)
    res = run_bass_kernel_spmd(nc, in_maps, core_ids=list(range(NCORES)))
    out = np.empty((B, N, D), np.float32)
    for c in range(NCORES):
        b, q = c >> 2, c & 3
        out[b, q * NQ:(q + 1) * NQ] = res.results[c]["out"].reshape(NQ, D)
    return out

